# revision 66
# baseline (speedup 1.0000x reference)
"""AnyStory Flux attention processor on 8 TRN2 NeuronCores.

Sharding: tensor-parallel over heads (24 heads -> 3 per core). No
collectives: each core computes full attention for its 3 heads; the host
gathers along the head axis.

Device algorithm per head (S=3168 = 512 txt + 64 redux + 2048 img +
512 ref + 32 router; D=128):
  seg1: q[0:2624] x k[0:3136] with additive mask, computed in S^T
        orientation (k on partitions, q on free axis) so every mask
        block is in its natural storage layout. No max-subtraction
        (logits bounded: |scaled logit| <~ 6, masks <= 1.5).
  seg2: per-cond ref self-attention (2 blocks of 256).
  seg3: router q (32) x [img keys ; router keys].
Softmax denominators come from a ones-column appended to V (PV matmul
accumulates [out | sum] in one PSUM region).
"""

import math
import numpy as np
import ml_dtypes
from contextlib import ExitStack

import concourse.bass as bass
import concourse.tile as tile
from concourse import mybir, bacc
from concourse.bass_utils import run_bass_kernel_spmd

# ---- problem constants (hardcoded; kernel.py must be self-contained)
B, H, D = 1, 24, 128
TXT, REDUX, IMG, REF, ROUTER, NCOND = 512, 64, 2048, 512, 32, 2
S = TXT + REDUX + IMG + REF + ROUTER          # 3168
TE = TXT                                       # 512
TRE = TE + REDUX                               # 576
TRI = TRE + IMG                                # 2624
TRIR = TRI + REF                               # 3136
REF_SHIFT = 1.5
SP = 3200                                      # padded key length (25*128)
NKT = SP // 128                                # 25 seg1 k-tiles
HPC = H // 8                                   # heads per core = 3
NEG = -1.0e4                                   # exp(NEG) == 0 in fp32

F32 = mybir.dt.float32
F32R = mybir.dt.float32r
BF16 = mybir.dt.bfloat16
EXP = mybir.ActivationFunctionType.Exp

# seg1 q blocks: 384 wide so 3 sub-blocks (3 x 129 cols) fit in one
# PSUM accumulator bank
QBLOCKS = [(0, 384), (384, 384), (768, 384), (1152, 384),
           (1536, 384), (1920, 384), (2304, 320)]
GROUP = 3                                      # k-tiles per PSUM/exp group
# group starts/sizes: avoid a ragged 1-tile tail group (its exp is shorter
# than the PE work window it must cover, stalling ACT at block boundaries)
KGROUPS = [(0, 3), (3, 3), (6, 3), (9, 3), (12, 3), (15, 3), (18, 3),
           (21, 2), (23, 2)]


def _subs(qw):
    """Split a q-block into <=128-row sub-blocks."""
    out, o = [], 0
    while o < qw:
        w = min(128, qw - o)
        out.append((o, w))
        o += w
    return out


def build_nc():
    nc = bacc.Bacc()
    qt_d = nc.declare_dram_parameter("qt", [HPC, 128, S], BF16, isOutput=False)
    kt_d = nc.declare_dram_parameter("kt", [HPC, 128, SP], BF16, isOutput=False)
    # V arrays are pre-tiled on host to partition-major [128, T, 129] so DMA
    # runs are ~6.4KB contiguous (sub-512B runs pay 2x DMA latency).
    v1_d = nc.declare_dram_parameter("v1", [HPC, 128, NKT, 129], BF16, isOutput=False)
    v2_d = nc.declare_dram_parameter("v2", [HPC, 128, 4, 129], BF16, isOutput=False)
    v3_d = nc.declare_dram_parameter("v3", [HPC, 128, 17, 129], BF16, isOutput=False)
    mref_d = nc.declare_dram_parameter("mref", [5, 128, TRI], BF16, isOutput=False)
    mrdx_d = nc.declare_dram_parameter("mredux", [64, TRI], BF16, isOutput=False)
    id_d = nc.declare_dram_parameter("ident", [128, 128], BF16, isOutput=False)
    out_d = nc.declare_dram_parameter("out", [HPC, S, 128], F32, isOutput=True)

    with ExitStack() as ctx:
        tc = ctx.enter_context(tile.TileContext(nc))
        const = ctx.enter_context(tc.tile_pool(name="const", bufs=1))
        stp = ctx.enter_context(tc.tile_pool(name="st", bufs=2, space="PSUM"))
        accp = ctx.enter_context(tc.tile_pool(name="acc", bufs=2, space="PSUM"))
        ptp = ctx.enter_context(tc.tile_pool(name="pt", bufs=6))
        smallp = ctx.enter_context(tc.tile_pool(name="small", bufs=8))

        # ---- persistent SBUF: per-head QT/KT/V tiles + masks
        kt_sb, qt_sb, v1_sb, v2_sb, v3_sb = [], [], [], [], []
        mref_sb = const.tile([128, 5, TRI], BF16, tag="mref")
        mrdx_sb = const.tile([64, TRI], BF16, tag="mredux")
        id_sb = const.tile([128, 128], BF16, tag="ident")
        for h in range(HPC):
            kt = const.tile([128, SP], BF16, tag=f"kt{h}")
            qt = const.tile([128, S], BF16, tag=f"qt{h}")
            v1 = const.tile([128, NKT, 129], BF16, tag=f"v1{h}")
            v2 = const.tile([128, 4, 129], BF16, tag=f"v2{h}")
            v3 = const.tile([128, 17, 129], BF16, tag=f"v3{h}")
            kt_sb.append(kt); qt_sb.append(qt); v1_sb.append(v1)
            v2_sb.append(v2); v3_sb.append(v3)
            if h == 0:
                # Head 0 gates startup: chunk the loads in the exact order
                # compute consumes them (k-tiles group by group, mask chunks
                # just before the masked tiles t>=20 are reached, later qt
                # columns just before their q-blocks start).
                def ktc(t0, t1):
                    nc.sync.dma_start(
                        kt[:, t0 * 128: t1 * 128], kt_d[h, :, t0 * 128: t1 * 128])

                def v1c(t0, t1):
                    nc.sync.dma_start(v1[:, t0:t1, :], v1_d[h, :, t0:t1, :])

                def qtc(c0, c1):
                    nc.sync.dma_start(qt[:, c0:c1], qt_d[h, :, c0:c1])

                def mrc(jj):
                    nc.sync.dma_start(mref_sb[:, jj, :], mref_d[jj])

                ktc(0, 3); qtc(0, 512)
                ktc(3, 6)
                nc.sync.dma_start(id_sb[:, :], id_d[:, :])
                nc.sync.dma_start(mrdx_sb[:, :], mrdx_d[:, :])
                v1c(0, 3); v1c(3, 6)
                ktc(6, 10); v1c(6, 10)
                qtc(512, 1024)
                ktc(10, 15); v1c(10, 15)
                ktc(15, 20); mrc(0); v1c(15, 20)
                ktc(20, 25); mrc(1); v1c(20, 25)
                mrc(2); mrc(3); mrc(4)
                qtc(1024, 2048); qtc(2048, S)
            else:
                # Chunked too: the DMA queue is effectively serial, so a
                # monolithic 4.5us load would head-of-line block the
                # latency-critical out-stores of the running head.
                for c in range(5):
                    t0, t1 = c * 5, min((c + 1) * 5, NKT)
                    nc.sync.dma_start(
                        kt[:, t0 * 128: t1 * 128], kt_d[h, :, t0 * 128: t1 * 128])
                    nc.sync.dma_start(v1[:, t0:t1, :], v1_d[h, :, t0:t1, :])
                for c in range(4):
                    c0, c1 = c * 792, (c + 1) * 792
                    nc.sync.dma_start(qt[:, c0:c1], qt_d[h, :, c0:c1])
            nc.sync.dma_start(v2[:, :, :], v2_d[h])
            nc.sync.dma_start(v3[:, :, :], v3_d[h])

        # ---- compute: one global 2-deep software pipeline over every
        # (head, segment, group) work item. PE stream is
        #   ..., QK_g, PV_{g-2}, QK_{g+1}, PV_{g-1}, ...
        # so PE never stalls on a just-finished exp, ACT runs back to back,
        # and the pipeline never drains at segment or head boundaries.
        items = []
        out_f = out_d.rearrange("h s d -> (h s) d")
        for h in range(HPC):
            kt, qt, v1 = kt_sb[h], qt_sb[h], v1_sb[h]
            ktr = kt[:]
            qtr = qt[:]

            def norm_out(acc, si, qsw, row0):
                rec = smallp.tile([128, 1], F32, tag="rec")
                nc.vector.reciprocal(
                    rec[0:qsw, :], acc[0:qsw, si * 129 + 128: si * 129 + 129])
                stg = smallp.tile([128, 128], F32, tag="stg")
                nc.vector.tensor_scalar_mul(
                    stg[0:qsw, :],
                    acc[0:qsw, si * 129: si * 129 + 128], rec[0:qsw, :])
                nc.sync.dma_start(out_f[row0: row0 + qsw, :], stg[0:qsw, :])

            # ===== seg1 items =====
            # Head 0 only: emit qb0/qb1's unmasked k-groups before their
            # masked ones so the first masked exp lands after the ~9us of
            # mask DMA has streamed in (mask matmuls commute -- PSUM adds).
            seg1_items = {}
            for qbi, (q0, qw) in enumerate(QBLOCKS):
                subs = _subs(qw)
                blk = {}  # lazily-allocated acc shared by the block's groups

                def qk1(st, h=h, ktr=ktr, qtr=qtr, q0=q0, qw=qw, g0=None,
                        ntile=None):
                    for j in range(ntile):
                        t = g0 + j
                        # Masked tiles: PE writes the mask into PSUM first
                        # (identity matmul, start=True clears the bank), then
                        # the QK matmul accumulates on top (start=False).
                        # Keeps DVE out of the exp critical chain.
                        masked = False
                        if t == 4:
                            # redux keys = partitions 0:64 of tile 4; mredux
                            # is zero-padded to the FULL [128, q] region --
                            # the mask matmul must cover the entire QK output
                            # region (partial-region PSUM init followed by an
                            # accumulating matmul corrupts the uncovered
                            # cells).
                            if q0 + qw > TRE:
                                # lhsT = identity rows 0:64 x all 128 cols:
                                # out rows 0:64 get the mask, rows 64:128 get
                                # an explicit 0 -- full-region coverage with
                                # half the mask bytes.
                                nc.tensor.matmul(
                                    st[:, j, 0:qw],
                                    lhsT=id_sb[0:64, :],
                                    rhs=mrdx_sb[:, q0:q0 + qw],
                                    start=True, stop=False)
                                masked = True
                        elif t >= 20:
                            nc.tensor.matmul(
                                st[:, j, 0:qw],
                                lhsT=id_sb[:, :],
                                rhs=mref_sb[:, t - 20, q0:q0 + qw],
                                start=True, stop=False)
                            masked = True
                        nc.tensor.matmul(
                            st[:, j, :qw],
                            lhsT=ktr[:, t * 128:(t + 1) * 128],
                            rhs=qtr[:, q0:q0 + qw],
                            start=not masked, stop=True)

                def ex1(st, pt, qw=qw, ntile=None):
                    nc.scalar.activation(
                        pt[:, 0:ntile, 0:qw], st[:, 0:ntile, 0:qw], EXP)

                def pv1(pt, h=h, v1=v1, q0=q0, qw=qw, subs=subs, blk=blk,
                        g0=None, ntile=None):
                    if "acc" not in blk:
                        blk["acc"] = accp.tile([128, 512], F32, tag="acc", name="acc")
                    acc = blk["acc"]
                    for j in range(ntile):
                        t = g0 + j
                        for si, (qs0, qsw) in enumerate(subs):
                            # start=True clears has_written for the WHOLE
                            # bank, so only the very first matmul into this
                            # bank sets it; other sub-regions overwrite fresh
                            # (their bits are clear) and accumulate after.
                            nc.tensor.matmul(
                                acc[0:qsw, si * 129: si * 129 + 129],
                                lhsT=pt[:, j, qs0:qs0 + qsw],
                                rhs=v1[:, t, :],
                                start=(t == 0 and si == 0),
                                stop=(t == NKT - 1))
                    if g0 + ntile >= NKT:
                        for si, (qs0, qsw) in enumerate(subs):
                            norm_out(acc, si, qsw, h * S + q0 + qs0)

                for (g0, ntile) in KGROUPS:
                    seg1_items[(qbi, g0)] = (
                        (lambda st, f=qk1, g0=g0, n=ntile: f(st, g0=g0, ntile=n)),
                        (lambda st, pt, f=ex1, n=ntile: f(st, pt, ntile=n)),
                        (lambda pt, f=pv1, g0=g0, n=ntile: f(pt, g0=g0, ntile=n)),
                    )
            if h == 0:
                order = ([(0, g) for (g, n) in KGROUPS[:6]]
                         + [(1, g) for (g, n) in KGROUPS[:6]]
                         + [(0, g) for (g, n) in KGROUPS[6:]]
                         + [(1, g) for (g, n) in KGROUPS[6:]]
                         + [(qbi, g) for qbi in range(2, len(QBLOCKS))
                            for (g, n) in KGROUPS])
            else:
                order = [(qbi, g) for qbi in range(len(QBLOCKS))
                         for (g, n) in KGROUPS]
            for key in order:
                items.append(seg1_items[key])

            # ===== seg2 items: per-cond ref self-attention =====
            for c in range(NCOND):
                b0 = TRI + 256 * c

                def qk2(st, ktr=ktr, qtr=qtr, b0=b0):
                    for j in range(2):
                        nc.tensor.matmul(
                            st[:, j, 0:256],
                            lhsT=ktr[:, b0 + j * 128: b0 + (j + 1) * 128],
                            rhs=qtr[:, b0: b0 + 256],
                            start=True, stop=True)

                def ex2(st, pt):
                    nc.scalar.activation(
                        pt[:, 0:2, 0:256], st[:, 0:2, 0:256], EXP)

                def pv2(pt, h=h, v2=v2_sb[h], b0=b0, c=c):
                    acc = accp.tile([128, 512], F32, tag="acc", name="acc")
                    for j in range(2):
                        for si in range(2):
                            nc.tensor.matmul(
                                acc[0:128, si * 129: si * 129 + 129],
                                lhsT=pt[:, j, si * 128:(si + 1) * 128],
                                rhs=v2[:, 2 * c + j, :],
                                start=(j == 0 and si == 0), stop=(j == 1))
                    for si in range(2):
                        norm_out(acc, si, 128, h * S + b0 + si * 128)

                items.append((qk2, ex2, pv2))

            # ===== seg3 item: router queries =====
            def qk3(st, ktr=ktr, qtr=qtr):
                for i in range(16):
                    nc.tensor.matmul(
                        st[:, 0, i * 32:(i + 1) * 32],
                        lhsT=ktr[:, TRE + i * 128: TRE + (i + 1) * 128],
                        rhs=qtr[:, TRIR: TRIR + 32],
                        start=True, stop=True)
                nc.tensor.matmul(
                    st[0:32, 1, 0:32],
                    lhsT=ktr[:, TRIR: TRIR + 32],
                    rhs=qtr[:, TRIR: TRIR + 32],
                    start=True, stop=True)

            def ex3(st, pt):
                nc.scalar.activation(pt[:, 0, 0:512], st[:, 0, 0:512], EXP)
                nc.scalar.activation(pt[0:32, 1, 0:32], st[0:32, 1, 0:32], EXP)

            def pv3(pt, h=h, v3=v3_sb[h]):
                acc = accp.tile([128, 512], F32, tag="acc", name="acc")
                for i in range(16):
                    nc.tensor.matmul(
                        acc[0:32, 0:129],
                        lhsT=pt[:, 0, i * 32:(i + 1) * 32],
                        rhs=v3[:, i, :],
                        start=(i == 0), stop=False)
                nc.tensor.matmul(
                    acc[0:32, 0:129],
                    lhsT=pt[0:32, 1, 0:32],
                    rhs=v3[0:32, 16, :],
                    start=False, stop=True)
                norm_out(acc, 0, 32, h * S + TRIR)

            items.append((qk3, ex3, pv3))

        # ---- run the global pipeline
        pending = []
        for (fqk, fex, fpv) in items:
            st = stp.tile([128, GROUP, 512], F32, tag="st", name="st")
            fqk(st)
            while len(pending) >= 2:
                pending.pop(0)[0]()
            pt = ptp.tile([128, GROUP, 512], BF16, tag="pt", name="pt")
            fex(st, pt)
            pending.append(((lambda f=fpv, p=pt: f(p)),))
        while pending:
            pending.pop(0)[0]()

    nc.compile()
    return nc


_NC_CACHE = None


def _get_nc():
    global _NC_CACHE
    if _NC_CACHE is None:
        _NC_CACHE = build_nc()
    return _NC_CACHE


def make_in_maps(query, key, value, ref_mask, routing_map):
    q = np.asarray(query, np.float32)[0] * (1.0 / math.sqrt(D))  # [24,S,128]
    k = np.asarray(key, np.float32)[0]
    v = np.asarray(value, np.float32)[0]
    qt = np.ascontiguousarray(q.transpose(0, 2, 1)).astype(
        ml_dtypes.bfloat16)                                      # [24,128,S]
    kt = np.zeros((H, 128, SP), np.float32)
    kt[:, :, :S] = k.transpose(0, 2, 1)
    kt = kt.astype(ml_dtypes.bfloat16)
    vv = np.zeros((H, SP, 129), np.float32)
    vv[:, :S, :128] = v
    vv[:, :S, 128] = 1.0
    vv = vv.astype(ml_dtypes.bfloat16)
    # pre-tiled partition-major V layouts: [H, 128, T, 129]
    v1 = np.ascontiguousarray(
        vv.reshape(H, NKT, 128, 129).transpose(0, 2, 1, 3))
    v2 = np.ascontiguousarray(
        vv[:, TRI:TRIR].reshape(H, 4, 128, 129).transpose(0, 2, 1, 3))
    v3 = np.zeros((H, 17, 128, 129), np.float32).astype(ml_dtypes.bfloat16)
    v3[:, 0:16] = vv[:, TRE:TRI].reshape(H, 16, 128, 129)
    v3[:, 16, 0:32] = vv[:, TRIR:TRIR + 32]
    v3 = np.ascontiguousarray(v3.transpose(0, 2, 1, 3))

    rm = np.asarray(ref_mask, np.float32)[0]                     # [512, 2624]
    rt = np.asarray(routing_map, np.float32)[0]                  # [2, 2048]
    base = (rm - 1.0) * 100.0 + REF_SHIFT
    ref_rt = np.repeat(rt, REF // NCOND, axis=0)                 # [512, 2048]
    base = base.copy()
    base[:, TRE:TRI] += (ref_rt - 1.0) * 100.0
    mref = np.zeros((5, 128, TRI), np.float32)
    mref[0, 64:128] = base[0:64]
    mref[1] = base[64:192]
    mref[2] = base[192:320]
    mref[3] = base[320:448]
    mref[4, 0:64] = base[448:512]
    mref[4, 64:128] = NEG                                        # kill router+pad
    mref = mref.astype(ml_dtypes.bfloat16)
    mredux = np.zeros((64, TRI), np.float32)
    mredux[:, TRE:TRI] = (np.repeat(rt, REDUX // NCOND, axis=0) - 1.0) * 100.0
    mredux = mredux.astype(ml_dtypes.bfloat16)

    ident = np.eye(128, dtype=np.float32).astype(ml_dtypes.bfloat16)
    in_maps = []
    for c in range(8):
        hs = slice(HPC * c, HPC * (c + 1))
        in_maps.append({
            "qt": np.ascontiguousarray(qt[hs]),
            "kt": np.ascontiguousarray(kt[hs]),
            "v1": np.ascontiguousarray(v1[hs]),
            "v2": np.ascontiguousarray(v2[hs]),
            "v3": np.ascontiguousarray(v3[hs]),
            "mref": mref,
            "mredux": np.ascontiguousarray(mredux),
            "ident": ident,
        })
    return in_maps


def kernel(query, key, value, ref_mask, routing_map, **_ignored):
    import jax
    if not any(d.platform == "axon" for d in jax.devices()):
        # the SPMD runner needs the 8 axon-tunneled NeuronCores visible
        jax.config.update("jax_platforms", "axon,cpu")
    nc = _get_nc()
    in_maps = make_in_maps(query, key, value, ref_mask, routing_map)
    res = run_bass_kernel_spmd(nc, in_maps, core_ids=list(range(8)))
    outs = [res.results[c]["out"] for c in range(8)]             # [3,S,128] each
    full = np.concatenate(outs, axis=0)[None]                    # [1,24,S,128]
    return np.ascontiguousarray(full.astype(np.float32))


# revision 69
# speedup vs baseline: 1.0037x; 1.0037x over previous
"""AnyStory Flux attention processor on 8 TRN2 NeuronCores.

Sharding: tensor-parallel over heads (24 heads -> 3 per core). No
collectives: each core computes full attention for its 3 heads; the host
gathers along the head axis.

Device algorithm per head (S=3168 = 512 txt + 64 redux + 2048 img +
512 ref + 32 router; D=128):
  seg1: q[0:2624] x k[0:3136] with additive mask, computed in S^T
        orientation (k on partitions, q on free axis) so every mask
        block is in its natural storage layout. No max-subtraction
        (logits bounded: |scaled logit| <~ 6, masks <= 1.5).
  seg2: per-cond ref self-attention (2 blocks of 256).
  seg3: router q (32) x [img keys ; router keys].
Softmax denominators come from a ones-column appended to V (PV matmul
accumulates [out | sum] in one PSUM region).
"""

import math
import numpy as np
import ml_dtypes
from contextlib import ExitStack

import concourse.bass as bass
import concourse.tile as tile
from concourse import mybir, bacc
from concourse.bass_utils import run_bass_kernel_spmd

# ---- problem constants (hardcoded; kernel.py must be self-contained)
B, H, D = 1, 24, 128
TXT, REDUX, IMG, REF, ROUTER, NCOND = 512, 64, 2048, 512, 32, 2
S = TXT + REDUX + IMG + REF + ROUTER          # 3168
TE = TXT                                       # 512
TRE = TE + REDUX                               # 576
TRI = TRE + IMG                                # 2624
TRIR = TRI + REF                               # 3136
REF_SHIFT = 1.5
SP = 3200                                      # padded key length (25*128)
NKT = SP // 128                                # 25 seg1 k-tiles
HPC = H // 8                                   # heads per core = 3
NEG = -1.0e4                                   # exp(NEG) == 0 in fp32

F32 = mybir.dt.float32
F32R = mybir.dt.float32r
BF16 = mybir.dt.bfloat16
EXP = mybir.ActivationFunctionType.Exp

# seg1 q blocks: 384 wide so 3 sub-blocks (3 x 129 cols) fit in one
# PSUM accumulator bank
QBLOCKS = [(0, 384), (384, 384), (768, 384), (1152, 384),
           (1536, 384), (1920, 384), (2304, 320)]
GROUP = 3                                      # k-tiles per PSUM/exp group
# group starts/sizes: avoid a ragged 1-tile tail group (its exp is shorter
# than the PE work window it must cover, stalling ACT at block boundaries)
KGROUPS = [(0, 3), (3, 3), (6, 3), (9, 3), (12, 3), (15, 3), (18, 3),
           (21, 2), (23, 2)]


def _subs(qw):
    """Split a q-block into <=128-row sub-blocks."""
    out, o = [], 0
    while o < qw:
        w = min(128, qw - o)
        out.append((o, w))
        o += w
    return out


def build_nc():
    nc = bacc.Bacc()
    qt_d = nc.declare_dram_parameter("qt", [HPC, 128, S], BF16, isOutput=False)
    kt_d = nc.declare_dram_parameter("kt", [HPC, 128, SP], BF16, isOutput=False)
    # V arrays are pre-tiled on host to partition-major [128, T, 129] so DMA
    # runs are ~6.4KB contiguous (sub-512B runs pay 2x DMA latency).
    v1_d = nc.declare_dram_parameter("v1", [HPC, 128, NKT, 129], BF16, isOutput=False)
    v2_d = nc.declare_dram_parameter("v2", [HPC, 128, 4, 129], BF16, isOutput=False)
    v3_d = nc.declare_dram_parameter("v3", [HPC, 128, 17, 129], BF16, isOutput=False)
    mref_d = nc.declare_dram_parameter("mref", [5, 128, TRI], BF16, isOutput=False)
    mrdx_d = nc.declare_dram_parameter("mredux", [64, TRI], BF16, isOutput=False)
    id_d = nc.declare_dram_parameter("ident", [128, 128], BF16, isOutput=False)
    out_d = nc.declare_dram_parameter("out", [HPC, S, 128], F32, isOutput=True)

    with ExitStack() as ctx:
        tc = ctx.enter_context(tile.TileContext(nc))
        const = ctx.enter_context(tc.tile_pool(name="const", bufs=1))
        stp = ctx.enter_context(tc.tile_pool(name="st", bufs=2, space="PSUM"))
        accp = ctx.enter_context(tc.tile_pool(name="acc", bufs=2, space="PSUM"))
        ptp = ctx.enter_context(tc.tile_pool(name="pt", bufs=6))
        smallp = ctx.enter_context(tc.tile_pool(name="small", bufs=8))

        # ---- persistent SBUF: per-head QT/KT/V tiles + masks
        kt_sb, qt_sb, v1_sb, v2_sb, v3_sb = [], [], [], [], []
        mref_sb = const.tile([128, 5, TRI], BF16, tag="mref")
        mrdx_sb = const.tile([64, TRI], BF16, tag="mredux")
        id_sb = const.tile([128, 128], BF16, tag="ident")
        for h in range(HPC):
            kt = const.tile([128, SP], BF16, tag=f"kt{h}")
            qt = const.tile([128, S], BF16, tag=f"qt{h}")
            v1 = const.tile([128, NKT, 129], BF16, tag=f"v1{h}")
            v2 = const.tile([128, 4, 129], BF16, tag=f"v2{h}")
            v3 = const.tile([128, 17, 129], BF16, tag=f"v3{h}")
            kt_sb.append(kt); qt_sb.append(qt); v1_sb.append(v1)
            v2_sb.append(v2); v3_sb.append(v3)
            if h == 0:
                # Head 0 gates startup: chunk the loads in the exact order
                # compute consumes them (k-tiles group by group, mask chunks
                # just before the masked tiles t>=20 are reached, later qt
                # columns just before their q-blocks start).
                def ktc(t0, t1):
                    nc.sync.dma_start(
                        kt[:, t0 * 128: t1 * 128], kt_d[h, :, t0 * 128: t1 * 128])

                def v1c(t0, t1):
                    nc.sync.dma_start(v1[:, t0:t1, :], v1_d[h, :, t0:t1, :])

                def qtc(c0, c1):
                    nc.sync.dma_start(qt[:, c0:c1], qt_d[h, :, c0:c1])

                def mrc(jj, c0=0, c1=TRI):
                    nc.sync.dma_start(
                        mref_sb[:, jj, c0:c1], mref_d[jj, :, c0:c1])

                # just-in-time order: each chunk lands right before the
                # pipeline consumes it (QK needs kt; the t=4 mask matmul in
                # group 1 needs id+mredux early; PV needs v1 two groups after
                # its exp; mref chunks feed the deferred masked groups)
                ktc(0, 3); qtc(0, 512); ktc(3, 6)
                nc.sync.dma_start(id_sb[:, :], id_d[:, :])
                nc.sync.dma_start(mrdx_sb[:, :], mrdx_d[:, :])
                ktc(6, 10); v1c(0, 3); v1c(3, 6)
                ktc(10, 15); v1c(6, 10)
                qtc(512, 1024)
                # early: only the mask columns qb0/qb1's deferred masked
                # groups read (q < 768); the rest follows after the qt loads
                ktc(15, 20); mrc(0, 0, 768); v1c(10, 15)
                ktc(20, 25); v1c(15, 20); mrc(1, 0, 768)
                mrc(2, 0, 768); v1c(20, 25); mrc(3, 0, 768); mrc(4, 0, 768)
                qtc(1024, 2048); qtc(2048, S)
                for jj in range(5):
                    mrc(jj, 768, TRI)
            else:
                # Chunked too: the DMA queue is effectively serial, so a
                # monolithic 4.5us load would head-of-line block the
                # latency-critical out-stores of the running head.
                for c in range(5):
                    t0, t1 = c * 5, min((c + 1) * 5, NKT)
                    nc.sync.dma_start(
                        kt[:, t0 * 128: t1 * 128], kt_d[h, :, t0 * 128: t1 * 128])
                    nc.sync.dma_start(v1[:, t0:t1, :], v1_d[h, :, t0:t1, :])
                for c in range(4):
                    c0, c1 = c * 792, (c + 1) * 792
                    nc.sync.dma_start(qt[:, c0:c1], qt_d[h, :, c0:c1])
            nc.sync.dma_start(v2[:, :, :], v2_d[h])
            nc.sync.dma_start(v3[:, :, :], v3_d[h])

        # ---- compute: one global 2-deep software pipeline over every
        # (head, segment, group) work item. PE stream is
        #   ..., QK_g, PV_{g-2}, QK_{g+1}, PV_{g-1}, ...
        # so PE never stalls on a just-finished exp, ACT runs back to back,
        # and the pipeline never drains at segment or head boundaries.
        items = []
        out_f = out_d.rearrange("h s d -> (h s) d")
        for h in range(HPC):
            kt, qt, v1 = kt_sb[h], qt_sb[h], v1_sb[h]
            ktr = kt[:]
            qtr = qt[:]

            def norm_out(acc, si, qsw, row0):
                rec = smallp.tile([128, 1], F32, tag="rec")
                nc.vector.reciprocal(
                    rec[0:qsw, :], acc[0:qsw, si * 129 + 128: si * 129 + 129])
                stg = smallp.tile([128, 128], F32, tag="stg")
                nc.vector.tensor_scalar_mul(
                    stg[0:qsw, :],
                    acc[0:qsw, si * 129: si * 129 + 128], rec[0:qsw, :])
                nc.sync.dma_start(out_f[row0: row0 + qsw, :], stg[0:qsw, :])

            # ===== seg1 items =====
            # Head 0 only: emit qb0/qb1's unmasked k-groups before their
            # masked ones so the first masked exp lands after the ~9us of
            # mask DMA has streamed in (mask matmuls commute -- PSUM adds).
            seg1_items = {}
            for qbi, (q0, qw) in enumerate(QBLOCKS):
                subs = _subs(qw)
                blk = {}  # lazily-allocated acc shared by the block's groups

                def qk1(st, h=h, ktr=ktr, qtr=qtr, q0=q0, qw=qw, g0=None,
                        ntile=None):
                    for j in range(ntile):
                        t = g0 + j
                        # Masked tiles: PE writes the mask into PSUM first
                        # (identity matmul, start=True clears the bank), then
                        # the QK matmul accumulates on top (start=False).
                        # Keeps DVE out of the exp critical chain.
                        masked = False
                        if t == 4:
                            # redux keys = partitions 0:64 of tile 4; mredux
                            # is zero-padded to the FULL [128, q] region --
                            # the mask matmul must cover the entire QK output
                            # region (partial-region PSUM init followed by an
                            # accumulating matmul corrupts the uncovered
                            # cells).
                            if q0 + qw > TRE:
                                # lhsT = identity rows 0:64 x all 128 cols:
                                # out rows 0:64 get the mask, rows 64:128 get
                                # an explicit 0 -- full-region coverage with
                                # half the mask bytes.
                                nc.tensor.matmul(
                                    st[:, j, 0:qw],
                                    lhsT=id_sb[0:64, :],
                                    rhs=mrdx_sb[:, q0:q0 + qw],
                                    start=True, stop=False)
                                masked = True
                        elif t >= 20:
                            nc.tensor.matmul(
                                st[:, j, 0:qw],
                                lhsT=id_sb[:, :],
                                rhs=mref_sb[:, t - 20, q0:q0 + qw],
                                start=True, stop=False)
                            masked = True
                        nc.tensor.matmul(
                            st[:, j, :qw],
                            lhsT=ktr[:, t * 128:(t + 1) * 128],
                            rhs=qtr[:, q0:q0 + qw],
                            start=not masked, stop=True)

                def ex1(st, pt, qw=qw, ntile=None):
                    nc.scalar.activation(
                        pt[:, 0:ntile, 0:qw], st[:, 0:ntile, 0:qw], EXP)

                def pv1(pt, h=h, v1=v1, q0=q0, qw=qw, subs=subs, blk=blk,
                        g0=None, ntile=None):
                    if "acc" not in blk:
                        blk["acc"] = accp.tile([128, 512], F32, tag="acc", name="acc")
                    acc = blk["acc"]
                    for j in range(ntile):
                        t = g0 + j
                        for si, (qs0, qsw) in enumerate(subs):
                            # start=True clears has_written for the WHOLE
                            # bank, so only the very first matmul into this
                            # bank sets it; other sub-regions overwrite fresh
                            # (their bits are clear) and accumulate after.
                            nc.tensor.matmul(
                                acc[0:qsw, si * 129: si * 129 + 129],
                                lhsT=pt[:, j, qs0:qs0 + qsw],
                                rhs=v1[:, t, :],
                                start=(t == 0 and si == 0),
                                stop=(t == NKT - 1))
                    if g0 + ntile >= NKT:
                        for si, (qs0, qsw) in enumerate(subs):
                            norm_out(acc, si, qsw, h * S + q0 + qs0)

                for (g0, ntile) in KGROUPS:
                    seg1_items[(qbi, g0)] = (
                        (lambda st, f=qk1, g0=g0, n=ntile: f(st, g0=g0, ntile=n)),
                        (lambda st, pt, f=ex1, n=ntile: f(st, pt, ntile=n)),
                        (lambda pt, f=pv1, g0=g0, n=ntile: f(pt, g0=g0, ntile=n)),
                    )
            if h == 0:
                order = ([(0, g) for (g, n) in KGROUPS[:6]]
                         + [(1, g) for (g, n) in KGROUPS[:6]]
                         + [(0, g) for (g, n) in KGROUPS[6:]]
                         + [(1, g) for (g, n) in KGROUPS[6:]]
                         + [(qbi, g) for qbi in range(2, len(QBLOCKS))
                            for (g, n) in KGROUPS])
            else:
                order = [(qbi, g) for qbi in range(len(QBLOCKS))
                         for (g, n) in KGROUPS]
            for key in order:
                items.append(seg1_items[key])

            # ===== seg2 items: per-cond ref self-attention =====
            for c in range(NCOND):
                b0 = TRI + 256 * c

                def qk2(st, ktr=ktr, qtr=qtr, b0=b0):
                    for j in range(2):
                        nc.tensor.matmul(
                            st[:, j, 0:256],
                            lhsT=ktr[:, b0 + j * 128: b0 + (j + 1) * 128],
                            rhs=qtr[:, b0: b0 + 256],
                            start=True, stop=True)

                def ex2(st, pt):
                    nc.scalar.activation(
                        pt[:, 0:2, 0:256], st[:, 0:2, 0:256], EXP)

                def pv2(pt, h=h, v2=v2_sb[h], b0=b0, c=c):
                    acc = accp.tile([128, 512], F32, tag="acc", name="acc")
                    for j in range(2):
                        for si in range(2):
                            nc.tensor.matmul(
                                acc[0:128, si * 129: si * 129 + 129],
                                lhsT=pt[:, j, si * 128:(si + 1) * 128],
                                rhs=v2[:, 2 * c + j, :],
                                start=(j == 0 and si == 0), stop=(j == 1))
                    for si in range(2):
                        norm_out(acc, si, 128, h * S + b0 + si * 128)

                items.append((qk2, ex2, pv2))

            # ===== seg3 item: router queries =====
            def qk3(st, ktr=ktr, qtr=qtr):
                for i in range(16):
                    nc.tensor.matmul(
                        st[:, 0, i * 32:(i + 1) * 32],
                        lhsT=ktr[:, TRE + i * 128: TRE + (i + 1) * 128],
                        rhs=qtr[:, TRIR: TRIR + 32],
                        start=True, stop=True)
                nc.tensor.matmul(
                    st[0:32, 1, 0:32],
                    lhsT=ktr[:, TRIR: TRIR + 32],
                    rhs=qtr[:, TRIR: TRIR + 32],
                    start=True, stop=True)

            def ex3(st, pt):
                nc.scalar.activation(pt[:, 0, 0:512], st[:, 0, 0:512], EXP)
                nc.scalar.activation(pt[0:32, 1, 0:32], st[0:32, 1, 0:32], EXP)

            def pv3(pt, h=h, v3=v3_sb[h]):
                acc = accp.tile([128, 512], F32, tag="acc", name="acc")
                for i in range(16):
                    nc.tensor.matmul(
                        acc[0:32, 0:129],
                        lhsT=pt[:, 0, i * 32:(i + 1) * 32],
                        rhs=v3[:, i, :],
                        start=(i == 0), stop=False)
                nc.tensor.matmul(
                    acc[0:32, 0:129],
                    lhsT=pt[0:32, 1, 0:32],
                    rhs=v3[0:32, 16, :],
                    start=False, stop=True)
                norm_out(acc, 0, 32, h * S + TRIR)

            items.append((qk3, ex3, pv3))

        # ---- run the global pipeline
        pending = []
        for (fqk, fex, fpv) in items:
            st = stp.tile([128, GROUP, 512], F32, tag="st", name="st")
            fqk(st)
            while len(pending) >= 2:
                pending.pop(0)[0]()
            pt = ptp.tile([128, GROUP, 512], BF16, tag="pt", name="pt")
            fex(st, pt)
            pending.append(((lambda f=fpv, p=pt: f(p)),))
        while pending:
            pending.pop(0)[0]()

    nc.compile()
    return nc


_NC_CACHE = None


def _get_nc():
    global _NC_CACHE
    if _NC_CACHE is None:
        _NC_CACHE = build_nc()
    return _NC_CACHE


def make_in_maps(query, key, value, ref_mask, routing_map):
    q = np.asarray(query, np.float32)[0] * (1.0 / math.sqrt(D))  # [24,S,128]
    k = np.asarray(key, np.float32)[0]
    v = np.asarray(value, np.float32)[0]
    qt = np.ascontiguousarray(q.transpose(0, 2, 1)).astype(
        ml_dtypes.bfloat16)                                      # [24,128,S]
    kt = np.zeros((H, 128, SP), np.float32)
    kt[:, :, :S] = k.transpose(0, 2, 1)
    kt = kt.astype(ml_dtypes.bfloat16)
    vv = np.zeros((H, SP, 129), np.float32)
    vv[:, :S, :128] = v
    vv[:, :S, 128] = 1.0
    vv = vv.astype(ml_dtypes.bfloat16)
    # pre-tiled partition-major V layouts: [H, 128, T, 129]
    v1 = np.ascontiguousarray(
        vv.reshape(H, NKT, 128, 129).transpose(0, 2, 1, 3))
    v2 = np.ascontiguousarray(
        vv[:, TRI:TRIR].reshape(H, 4, 128, 129).transpose(0, 2, 1, 3))
    v3 = np.zeros((H, 17, 128, 129), np.float32).astype(ml_dtypes.bfloat16)
    v3[:, 0:16] = vv[:, TRE:TRI].reshape(H, 16, 128, 129)
    v3[:, 16, 0:32] = vv[:, TRIR:TRIR + 32]
    v3 = np.ascontiguousarray(v3.transpose(0, 2, 1, 3))

    rm = np.asarray(ref_mask, np.float32)[0]                     # [512, 2624]
    rt = np.asarray(routing_map, np.float32)[0]                  # [2, 2048]
    base = (rm - 1.0) * 100.0 + REF_SHIFT
    ref_rt = np.repeat(rt, REF // NCOND, axis=0)                 # [512, 2048]
    base = base.copy()
    base[:, TRE:TRI] += (ref_rt - 1.0) * 100.0
    mref = np.zeros((5, 128, TRI), np.float32)
    mref[0, 64:128] = base[0:64]
    mref[1] = base[64:192]
    mref[2] = base[192:320]
    mref[3] = base[320:448]
    mref[4, 0:64] = base[448:512]
    mref[4, 64:128] = NEG                                        # kill router+pad
    mref = mref.astype(ml_dtypes.bfloat16)
    mredux = np.zeros((64, TRI), np.float32)
    mredux[:, TRE:TRI] = (np.repeat(rt, REDUX // NCOND, axis=0) - 1.0) * 100.0
    mredux = mredux.astype(ml_dtypes.bfloat16)

    ident = np.eye(128, dtype=np.float32).astype(ml_dtypes.bfloat16)
    in_maps = []
    for c in range(8):
        hs = slice(HPC * c, HPC * (c + 1))
        in_maps.append({
            "qt": np.ascontiguousarray(qt[hs]),
            "kt": np.ascontiguousarray(kt[hs]),
            "v1": np.ascontiguousarray(v1[hs]),
            "v2": np.ascontiguousarray(v2[hs]),
            "v3": np.ascontiguousarray(v3[hs]),
            "mref": mref,
            "mredux": np.ascontiguousarray(mredux),
            "ident": ident,
        })
    return in_maps


def kernel(query, key, value, ref_mask, routing_map, **_ignored):
    import jax
    if not any(d.platform == "axon" for d in jax.devices()):
        # the SPMD runner needs the 8 axon-tunneled NeuronCores visible
        jax.config.update("jax_platforms", "axon,cpu")
    nc = _get_nc()
    in_maps = make_in_maps(query, key, value, ref_mask, routing_map)
    res = run_bass_kernel_spmd(nc, in_maps, core_ids=list(range(8)))
    outs = [res.results[c]["out"] for c in range(8)]             # [3,S,128] each
    full = np.concatenate(outs, axis=0)[None]                    # [1,24,S,128]
    return np.ascontiguousarray(full.astype(np.float32))


# revision 72
# speedup vs baseline: 1.0196x; 1.0158x over previous
"""AnyStory Flux attention processor on 8 TRN2 NeuronCores.

Sharding: tensor-parallel over heads (24 heads -> 3 per core). No
collectives: each core computes full attention for its 3 heads; the host
gathers along the head axis.

Device algorithm per head (S=3168 = 512 txt + 64 redux + 2048 img +
512 ref + 32 router; D=128):
  seg1: q[0:2624] x k[0:3136] with additive mask, computed in S^T
        orientation (k on partitions, q on free axis) so every mask
        block is in its natural storage layout. No max-subtraction
        (logits bounded: |scaled logit| <~ 6, masks <= 1.5).
  seg2: per-cond ref self-attention (2 blocks of 256).
  seg3: router q (32) x [img keys ; router keys].
Softmax denominators come from a ones-column appended to V (PV matmul
accumulates [out | sum] in one PSUM region).
"""

import math
import numpy as np
import ml_dtypes
from contextlib import ExitStack

import concourse.bass as bass
import concourse.tile as tile
from concourse import mybir, bacc
from concourse.bass_utils import run_bass_kernel_spmd

# ---- problem constants (hardcoded; kernel.py must be self-contained)
B, H, D = 1, 24, 128
TXT, REDUX, IMG, REF, ROUTER, NCOND = 512, 64, 2048, 512, 32, 2
S = TXT + REDUX + IMG + REF + ROUTER          # 3168
TE = TXT                                       # 512
TRE = TE + REDUX                               # 576
TRI = TRE + IMG                                # 2624
TRIR = TRI + REF                               # 3136
REF_SHIFT = 1.5
SP = 3200                                      # padded key length (25*128)
NKT = SP // 128                                # 25 seg1 k-tiles
HPC = H // 8                                   # heads per core = 3
NEG = -1.0e4                                   # exp(NEG) == 0 in fp32

F32 = mybir.dt.float32
F32R = mybir.dt.float32r
BF16 = mybir.dt.bfloat16
EXP = mybir.ActivationFunctionType.Exp

# seg1 q blocks: 384 wide so 3 sub-blocks (3 x 129 cols) fit in one
# PSUM accumulator bank
QBLOCKS = [(0, 384), (384, 384), (768, 384), (1152, 384),
           (1536, 384), (1920, 384), (2304, 320)]
GROUP = 3                                      # k-tiles per PSUM/exp group
# k-tile groups (need NOT be contiguous): masked tiles (4, 20..24) are
# spread so every group carries at most ONE mask matmul -- keeps each
# group's PE window (QK + mask + PV) below its exp duration, so ACT never
# waits. No 1-tile tail group (its exp would be shorter than the PE window).
KGROUPS = [(0, 1, 2), (3, 4, 5), (6, 7, 8), (9, 10, 11), (12, 13, 20),
           (14, 15, 21), (16, 17, 22), (18, 23), (19, 24)]


def _subs(qw):
    """Split a q-block into <=128-row sub-blocks."""
    out, o = [], 0
    while o < qw:
        w = min(128, qw - o)
        out.append((o, w))
        o += w
    return out


def build_nc():
    nc = bacc.Bacc()
    qt_d = nc.declare_dram_parameter("qt", [HPC, 128, S], BF16, isOutput=False)
    kt_d = nc.declare_dram_parameter("kt", [HPC, 128, SP], BF16, isOutput=False)
    # V arrays are pre-tiled on host to partition-major [128, T, 129] so DMA
    # runs are ~6.4KB contiguous (sub-512B runs pay 2x DMA latency).
    v1_d = nc.declare_dram_parameter("v1", [HPC, 128, NKT, 129], BF16, isOutput=False)
    v2_d = nc.declare_dram_parameter("v2", [HPC, 128, 4, 129], BF16, isOutput=False)
    v3_d = nc.declare_dram_parameter("v3", [HPC, 128, 17, 129], BF16, isOutput=False)
    mref_d = nc.declare_dram_parameter("mref", [5, 128, TRI], BF16, isOutput=False)
    mrdx_d = nc.declare_dram_parameter("mredux", [64, TRI], BF16, isOutput=False)
    id_d = nc.declare_dram_parameter("ident", [128, 128], BF16, isOutput=False)
    out_d = nc.declare_dram_parameter("out", [HPC, S, 128], F32, isOutput=True)

    with ExitStack() as ctx:
        tc = ctx.enter_context(tile.TileContext(nc))
        const = ctx.enter_context(tc.tile_pool(name="const", bufs=1))
        stp = ctx.enter_context(tc.tile_pool(name="st", bufs=2, space="PSUM"))
        accp = ctx.enter_context(tc.tile_pool(name="acc", bufs=2, space="PSUM"))
        ptp = ctx.enter_context(tc.tile_pool(name="pt", bufs=6))
        smallp = ctx.enter_context(tc.tile_pool(name="small", bufs=8))

        # ---- persistent SBUF: per-head QT/KT/V tiles + masks
        kt_sb, qt_sb, v1_sb, v2_sb, v3_sb = [], [], [], [], []
        mref_sb = const.tile([128, 5, TRI], BF16, tag="mref")
        mrdx_sb = const.tile([64, TRI], BF16, tag="mredux")
        id_sb = const.tile([128, 128], BF16, tag="ident")
        for h in range(HPC):
            kt = const.tile([128, SP], BF16, tag=f"kt{h}")
            qt = const.tile([128, S], BF16, tag=f"qt{h}")
            v1 = const.tile([128, NKT, 129], BF16, tag=f"v1{h}")
            v2 = const.tile([128, 4, 129], BF16, tag=f"v2{h}")
            v3 = const.tile([128, 17, 129], BF16, tag=f"v3{h}")
            kt_sb.append(kt); qt_sb.append(qt); v1_sb.append(v1)
            v2_sb.append(v2); v3_sb.append(v3)
            if h == 0:
                # Head 0 gates startup: chunk the loads in the exact order
                # compute consumes them (k-tiles group by group, mask chunks
                # just before the masked tiles t>=20 are reached, later qt
                # columns just before their q-blocks start).
                def ktc(t0, t1):
                    nc.sync.dma_start(
                        kt[:, t0 * 128: t1 * 128], kt_d[h, :, t0 * 128: t1 * 128])

                def v1c(t0, t1):
                    nc.sync.dma_start(v1[:, t0:t1, :], v1_d[h, :, t0:t1, :])

                def qtc(c0, c1):
                    nc.sync.dma_start(qt[:, c0:c1], qt_d[h, :, c0:c1])

                def mrc(jj, c0=0, c1=TRI):
                    nc.sync.dma_start(
                        mref_sb[:, jj, c0:c1], mref_d[jj, :, c0:c1])

                # just-in-time order: each chunk lands right before the
                # pipeline consumes it (QK needs kt; the t=4 mask matmul in
                # group 1 needs id+mredux early; PV needs v1 two groups after
                # its exp; mref chunks feed the deferred masked groups)
                ktc(0, 3); qtc(0, 512); ktc(3, 6)
                nc.sync.dma_start(id_sb[:, :], id_d[:, :])
                nc.sync.dma_start(mrdx_sb[:, :], mrdx_d[:, :])
                ktc(6, 10); v1c(0, 3); v1c(3, 6)
                ktc(10, 15); v1c(6, 10)
                qtc(512, 1024)
                # early: only the mask columns qb0/qb1's deferred masked
                # groups read (q < 768); the rest follows after the qt loads
                ktc(15, 20); mrc(0, 0, 768); v1c(10, 15)
                ktc(20, 25); v1c(15, 20); mrc(1, 0, 768)
                mrc(2, 0, 768); v1c(20, 25); mrc(3, 0, 768); mrc(4, 0, 768)
                qtc(1024, 2048); qtc(2048, S)
                for jj in range(5):
                    mrc(jj, 768, TRI)
            else:
                # Chunked too: the DMA queue is effectively serial, so a
                # monolithic 4.5us load would head-of-line block the
                # latency-critical out-stores of the running head.
                for c in range(5):
                    t0, t1 = c * 5, min((c + 1) * 5, NKT)
                    nc.sync.dma_start(
                        kt[:, t0 * 128: t1 * 128], kt_d[h, :, t0 * 128: t1 * 128])
                    nc.sync.dma_start(v1[:, t0:t1, :], v1_d[h, :, t0:t1, :])
                for c in range(4):
                    c0, c1 = c * 792, (c + 1) * 792
                    nc.sync.dma_start(qt[:, c0:c1], qt_d[h, :, c0:c1])
            nc.sync.dma_start(v2[:, :, :], v2_d[h])
            nc.sync.dma_start(v3[:, :, :], v3_d[h])

        # ---- compute: one global 2-deep software pipeline over every
        # (head, segment, group) work item. PE stream is
        #   ..., QK_g, PV_{g-2}, QK_{g+1}, PV_{g-1}, ...
        # so PE never stalls on a just-finished exp, ACT runs back to back,
        # and the pipeline never drains at segment or head boundaries.
        items = []
        out_f = out_d.rearrange("h s d -> (h s) d")
        for h in range(HPC):
            kt, qt, v1 = kt_sb[h], qt_sb[h], v1_sb[h]
            ktr = kt[:]
            qtr = qt[:]

            def norm_out(acc, si, qsw, row0):
                rec = smallp.tile([128, 1], F32, tag="rec")
                nc.vector.reciprocal(
                    rec[0:qsw, :], acc[0:qsw, si * 129 + 128: si * 129 + 129])
                stg = smallp.tile([128, 128], F32, tag="stg")
                nc.vector.tensor_scalar_mul(
                    stg[0:qsw, :],
                    acc[0:qsw, si * 129: si * 129 + 128], rec[0:qsw, :])
                nc.sync.dma_start(out_f[row0: row0 + qsw, :], stg[0:qsw, :])

            # ===== seg1 items =====
            # Head 0 only: emit qb0/qb1's unmasked k-groups before their
            # masked ones so the first masked exp lands after the ~9us of
            # mask DMA has streamed in (mask matmuls commute -- PSUM adds).
            seg1_items = {}
            for qbi, (q0, qw) in enumerate(QBLOCKS):
                subs = _subs(qw)
                blk = {}  # lazily-allocated acc shared by the block's groups

                def qk1(st, h=h, ktr=ktr, qtr=qtr, q0=q0, qw=qw,
                        tiles=None):
                    for j, t in enumerate(tiles):
                        # Masked tiles: PE writes the mask into PSUM first
                        # (identity matmul, start=True clears the bank), then
                        # the QK matmul accumulates on top (start=False).
                        # Keeps DVE out of the exp critical chain.
                        masked = False
                        if t == 4:
                            # redux keys = partitions 0:64 of tile 4; mredux
                            # is zero-padded to the FULL [128, q] region --
                            # the mask matmul must cover the entire QK output
                            # region (partial-region PSUM init followed by an
                            # accumulating matmul corrupts the uncovered
                            # cells).
                            if q0 + qw > TRE:
                                # lhsT = identity rows 0:64 x all 128 cols:
                                # out rows 0:64 get the mask, rows 64:128 get
                                # an explicit 0 -- full-region coverage with
                                # half the mask bytes.
                                nc.tensor.matmul(
                                    st[:, j, 0:qw],
                                    lhsT=id_sb[0:64, :],
                                    rhs=mrdx_sb[:, q0:q0 + qw],
                                    start=True, stop=False)
                                masked = True
                        elif t >= 20:
                            nc.tensor.matmul(
                                st[:, j, 0:qw],
                                lhsT=id_sb[:, :],
                                rhs=mref_sb[:, t - 20, q0:q0 + qw],
                                start=True, stop=False)
                            masked = True
                        nc.tensor.matmul(
                            st[:, j, :qw],
                            lhsT=ktr[:, t * 128:(t + 1) * 128],
                            rhs=qtr[:, q0:q0 + qw],
                            start=not masked, stop=True)

                def ex1(st, pt, qw=qw, ntile=None):
                    nc.scalar.activation(
                        pt[:, 0:ntile, 0:qw], st[:, 0:ntile, 0:qw], EXP)

                def pv1(pt, h=h, v1=v1, q0=q0, qw=qw, subs=subs, blk=blk,
                        tiles=None):
                    if "acc" not in blk:
                        blk["acc"] = accp.tile([128, 512], F32, tag="acc", name="acc")
                    acc = blk["acc"]
                    for j, t in enumerate(tiles):
                        for si, (qs0, qsw) in enumerate(subs):
                            # start=True clears has_written for the WHOLE
                            # bank, so only the very first matmul into this
                            # bank sets it; other sub-regions overwrite fresh
                            # (their bits are clear) and accumulate after.
                            nc.tensor.matmul(
                                acc[0:qsw, si * 129: si * 129 + 129],
                                lhsT=pt[:, j, qs0:qs0 + qsw],
                                rhs=v1[:, t, :],
                                start=(t == 0 and si == 0),
                                stop=(t == NKT - 1))
                    if NKT - 1 in tiles:
                        for si, (qs0, qsw) in enumerate(subs):
                            norm_out(acc, si, qsw, h * S + q0 + qs0)

                for gi, tiles in enumerate(KGROUPS):
                    seg1_items[(qbi, gi)] = (
                        (lambda st, f=qk1, tl=tiles: f(st, tiles=tl)),
                        (lambda st, pt, f=ex1, n=len(tiles): f(st, pt, ntile=n)),
                        (lambda pt, f=pv1, tl=tiles: f(pt, tiles=tl)),
                    )
            if h == 0:
                # groups 0-3 touch no mref-masked tile; defer the rest of
                # qb0/qb1 until the ref-mask chunks have streamed in
                order = ([(0, gi) for gi in range(4)]
                         + [(1, gi) for gi in range(4)]
                         + [(0, gi) for gi in range(4, 9)]
                         + [(1, gi) for gi in range(4, 9)]
                         + [(qbi, gi) for qbi in range(2, len(QBLOCKS))
                            for gi in range(9)])
            else:
                order = [(qbi, gi) for qbi in range(len(QBLOCKS))
                         for gi in range(9)]
            for key in order:
                items.append(seg1_items[key])

            # ===== seg2 items: per-cond ref self-attention =====
            for c in range(NCOND):
                b0 = TRI + 256 * c

                def qk2(st, ktr=ktr, qtr=qtr, b0=b0):
                    for j in range(2):
                        nc.tensor.matmul(
                            st[:, j, 0:256],
                            lhsT=ktr[:, b0 + j * 128: b0 + (j + 1) * 128],
                            rhs=qtr[:, b0: b0 + 256],
                            start=True, stop=True)

                def ex2(st, pt):
                    nc.scalar.activation(
                        pt[:, 0:2, 0:256], st[:, 0:2, 0:256], EXP)

                def pv2(pt, h=h, v2=v2_sb[h], b0=b0, c=c):
                    acc = accp.tile([128, 512], F32, tag="acc", name="acc")
                    for j in range(2):
                        for si in range(2):
                            nc.tensor.matmul(
                                acc[0:128, si * 129: si * 129 + 129],
                                lhsT=pt[:, j, si * 128:(si + 1) * 128],
                                rhs=v2[:, 2 * c + j, :],
                                start=(j == 0 and si == 0), stop=(j == 1))
                    for si in range(2):
                        norm_out(acc, si, 128, h * S + b0 + si * 128)

                items.append((qk2, ex2, pv2))

            # ===== seg3 item: router queries =====
            def qk3(st, ktr=ktr, qtr=qtr):
                for i in range(16):
                    nc.tensor.matmul(
                        st[:, 0, i * 32:(i + 1) * 32],
                        lhsT=ktr[:, TRE + i * 128: TRE + (i + 1) * 128],
                        rhs=qtr[:, TRIR: TRIR + 32],
                        start=True, stop=True)
                nc.tensor.matmul(
                    st[0:32, 1, 0:32],
                    lhsT=ktr[:, TRIR: TRIR + 32],
                    rhs=qtr[:, TRIR: TRIR + 32],
                    start=True, stop=True)

            def ex3(st, pt):
                nc.scalar.activation(pt[:, 0, 0:512], st[:, 0, 0:512], EXP)
                nc.scalar.activation(pt[0:32, 1, 0:32], st[0:32, 1, 0:32], EXP)

            def pv3(pt, h=h, v3=v3_sb[h]):
                acc = accp.tile([128, 512], F32, tag="acc", name="acc")
                for i in range(16):
                    nc.tensor.matmul(
                        acc[0:32, 0:129],
                        lhsT=pt[:, 0, i * 32:(i + 1) * 32],
                        rhs=v3[:, i, :],
                        start=(i == 0), stop=False)
                nc.tensor.matmul(
                    acc[0:32, 0:129],
                    lhsT=pt[0:32, 1, 0:32],
                    rhs=v3[0:32, 16, :],
                    start=False, stop=True)
                norm_out(acc, 0, 32, h * S + TRIR)

            items.append((qk3, ex3, pv3))

        # ---- run the global pipeline
        pending = []
        for (fqk, fex, fpv) in items:
            st = stp.tile([128, GROUP, 512], F32, tag="st", name="st")
            fqk(st)
            while len(pending) >= 2:
                pending.pop(0)[0]()
            pt = ptp.tile([128, GROUP, 512], BF16, tag="pt", name="pt")
            fex(st, pt)
            pending.append(((lambda f=fpv, p=pt: f(p)),))
        while pending:
            pending.pop(0)[0]()

    nc.compile()
    return nc


_NC_CACHE = None


def _get_nc():
    global _NC_CACHE
    if _NC_CACHE is None:
        _NC_CACHE = build_nc()
    return _NC_CACHE


def make_in_maps(query, key, value, ref_mask, routing_map):
    q = np.asarray(query, np.float32)[0] * (1.0 / math.sqrt(D))  # [24,S,128]
    k = np.asarray(key, np.float32)[0]
    v = np.asarray(value, np.float32)[0]
    qt = np.ascontiguousarray(q.transpose(0, 2, 1)).astype(
        ml_dtypes.bfloat16)                                      # [24,128,S]
    kt = np.zeros((H, 128, SP), np.float32)
    kt[:, :, :S] = k.transpose(0, 2, 1)
    kt = kt.astype(ml_dtypes.bfloat16)
    vv = np.zeros((H, SP, 129), np.float32)
    vv[:, :S, :128] = v
    vv[:, :S, 128] = 1.0
    vv = vv.astype(ml_dtypes.bfloat16)
    # pre-tiled partition-major V layouts: [H, 128, T, 129]
    v1 = np.ascontiguousarray(
        vv.reshape(H, NKT, 128, 129).transpose(0, 2, 1, 3))
    v2 = np.ascontiguousarray(
        vv[:, TRI:TRIR].reshape(H, 4, 128, 129).transpose(0, 2, 1, 3))
    v3 = np.zeros((H, 17, 128, 129), np.float32).astype(ml_dtypes.bfloat16)
    v3[:, 0:16] = vv[:, TRE:TRI].reshape(H, 16, 128, 129)
    v3[:, 16, 0:32] = vv[:, TRIR:TRIR + 32]
    v3 = np.ascontiguousarray(v3.transpose(0, 2, 1, 3))

    rm = np.asarray(ref_mask, np.float32)[0]                     # [512, 2624]
    rt = np.asarray(routing_map, np.float32)[0]                  # [2, 2048]
    base = (rm - 1.0) * 100.0 + REF_SHIFT
    ref_rt = np.repeat(rt, REF // NCOND, axis=0)                 # [512, 2048]
    base = base.copy()
    base[:, TRE:TRI] += (ref_rt - 1.0) * 100.0
    mref = np.zeros((5, 128, TRI), np.float32)
    mref[0, 64:128] = base[0:64]
    mref[1] = base[64:192]
    mref[2] = base[192:320]
    mref[3] = base[320:448]
    mref[4, 0:64] = base[448:512]
    mref[4, 64:128] = NEG                                        # kill router+pad
    mref = mref.astype(ml_dtypes.bfloat16)
    mredux = np.zeros((64, TRI), np.float32)
    mredux[:, TRE:TRI] = (np.repeat(rt, REDUX // NCOND, axis=0) - 1.0) * 100.0
    mredux = mredux.astype(ml_dtypes.bfloat16)

    ident = np.eye(128, dtype=np.float32).astype(ml_dtypes.bfloat16)
    in_maps = []
    for c in range(8):
        hs = slice(HPC * c, HPC * (c + 1))
        in_maps.append({
            "qt": np.ascontiguousarray(qt[hs]),
            "kt": np.ascontiguousarray(kt[hs]),
            "v1": np.ascontiguousarray(v1[hs]),
            "v2": np.ascontiguousarray(v2[hs]),
            "v3": np.ascontiguousarray(v3[hs]),
            "mref": mref,
            "mredux": np.ascontiguousarray(mredux),
            "ident": ident,
        })
    return in_maps


def kernel(query, key, value, ref_mask, routing_map, **_ignored):
    import jax
    if not any(d.platform == "axon" for d in jax.devices()):
        # the SPMD runner needs the 8 axon-tunneled NeuronCores visible
        jax.config.update("jax_platforms", "axon,cpu")
    nc = _get_nc()
    in_maps = make_in_maps(query, key, value, ref_mask, routing_map)
    res = run_bass_kernel_spmd(nc, in_maps, core_ids=list(range(8)))
    outs = [res.results[c]["out"] for c in range(8)]             # [3,S,128] each
    full = np.concatenate(outs, axis=0)[None]                    # [1,24,S,128]
    return np.ascontiguousarray(full.astype(np.float32))


# revision 75
# speedup vs baseline: 1.0208x; 1.0012x over previous
"""AnyStory Flux attention processor on 8 TRN2 NeuronCores.

Sharding: tensor-parallel over heads (24 heads -> 3 per core). No
collectives: each core computes full attention for its 3 heads; the host
gathers along the head axis.

Device algorithm per head (S=3168 = 512 txt + 64 redux + 2048 img +
512 ref + 32 router; D=128):
  seg1: q[0:2624] x k[0:3136] with additive mask, computed in S^T
        orientation (k on partitions, q on free axis) so every mask
        block is in its natural storage layout. No max-subtraction
        (logits bounded: |scaled logit| <~ 6, masks <= 1.5).
  seg2: per-cond ref self-attention (2 blocks of 256).
  seg3: router q (32) x [img keys ; router keys].
Softmax denominators come from a ones-column appended to V (PV matmul
accumulates [out | sum] in one PSUM region).
"""

import math
import numpy as np
import ml_dtypes
from contextlib import ExitStack

import concourse.bass as bass
import concourse.tile as tile
from concourse import mybir, bacc
from concourse.bass_utils import run_bass_kernel_spmd

# ---- problem constants (hardcoded; kernel.py must be self-contained)
B, H, D = 1, 24, 128
TXT, REDUX, IMG, REF, ROUTER, NCOND = 512, 64, 2048, 512, 32, 2
S = TXT + REDUX + IMG + REF + ROUTER          # 3168
TE = TXT                                       # 512
TRE = TE + REDUX                               # 576
TRI = TRE + IMG                                # 2624
TRIR = TRI + REF                               # 3136
REF_SHIFT = 1.5
SP = 3200                                      # padded key length (25*128)
NKT = SP // 128                                # 25 seg1 k-tiles
HPC = H // 8                                   # heads per core = 3
NEG = -1.0e4                                   # exp(NEG) == 0 in fp32

F32 = mybir.dt.float32
F32R = mybir.dt.float32r
BF16 = mybir.dt.bfloat16
EXP = mybir.ActivationFunctionType.Exp

# seg1 q blocks: 384 wide so 3 sub-blocks (3 x 129 cols) fit in one
# PSUM accumulator bank
QBLOCKS = [(0, 384), (384, 384), (768, 384), (1152, 384),
           (1536, 384), (1920, 384), (2304, 320)]
GROUP = 3                                      # k-tiles per PSUM/exp group
# k-tile groups (need NOT be contiguous): masked tiles (4, 20..24) are
# spread so every group carries at most ONE mask matmul -- keeps each
# group's PE window (QK + mask + PV) below its exp duration, so ACT never
# waits. No 1-tile tail group (its exp would be shorter than the PE window).
KGROUPS = [(0, 1, 2), (3, 4, 5), (6, 7, 8), (9, 10, 11), (12, 13, 20),
           (14, 15, 21), (16, 17, 22), (18, 23), (19, 24)]


def _subs(qw):
    """Split a q-block into <=128-row sub-blocks."""
    out, o = [], 0
    while o < qw:
        w = min(128, qw - o)
        out.append((o, w))
        o += w
    return out


def build_nc():
    nc = bacc.Bacc()
    qt_d = nc.declare_dram_parameter("qt", [HPC, 128, S], BF16, isOutput=False)
    kt_d = nc.declare_dram_parameter("kt", [HPC, 128, SP], BF16, isOutput=False)
    # V arrays are pre-tiled on host to partition-major [128, T, 129] so DMA
    # runs are ~6.4KB contiguous (sub-512B runs pay 2x DMA latency).
    v1_d = nc.declare_dram_parameter("v1", [HPC, 128, NKT, 129], BF16, isOutput=False)
    v2_d = nc.declare_dram_parameter("v2", [HPC, 128, 4, 129], BF16, isOutput=False)
    v3_d = nc.declare_dram_parameter("v3", [HPC, 128, 17, 129], BF16, isOutput=False)
    mref_d = nc.declare_dram_parameter("mref", [5, 128, TRI], BF16, isOutput=False)
    mrdx_d = nc.declare_dram_parameter("mredux", [64, TRI], BF16, isOutput=False)
    id_d = nc.declare_dram_parameter("ident", [128, 128], BF16, isOutput=False)
    out_d = nc.declare_dram_parameter("out", [HPC, S, 128], F32, isOutput=True)

    with ExitStack() as ctx:
        tc = ctx.enter_context(tile.TileContext(nc))
        const = ctx.enter_context(tc.tile_pool(name="const", bufs=1))
        stp = ctx.enter_context(tc.tile_pool(name="st", bufs=2, space="PSUM"))
        accp = ctx.enter_context(tc.tile_pool(name="acc", bufs=2, space="PSUM"))
        ptp = ctx.enter_context(tc.tile_pool(name="pt", bufs=6))
        smallp = ctx.enter_context(tc.tile_pool(name="small", bufs=8))

        # ---- persistent SBUF: per-head QT/KT/V tiles + masks
        kt_sb, qt_sb, v1_sb, v2_sb, v3_sb = [], [], [], [], []
        mref_sb = const.tile([128, 5, TRI], BF16, tag="mref")
        mrdx_sb = const.tile([64, TRI], BF16, tag="mredux")
        id_sb = const.tile([128, 128], BF16, tag="ident")
        for h in range(HPC):
            kt = const.tile([128, SP], BF16, tag=f"kt{h}")
            qt = const.tile([128, S], BF16, tag=f"qt{h}")
            v1 = const.tile([128, NKT, 129], BF16, tag=f"v1{h}")
            v2 = const.tile([128, 4, 129], BF16, tag=f"v2{h}")
            v3 = const.tile([128, 17, 129], BF16, tag=f"v3{h}")
            kt_sb.append(kt); qt_sb.append(qt); v1_sb.append(v1)
            v2_sb.append(v2); v3_sb.append(v3)
            if h == 0:
                # Head 0 gates startup: chunk the loads in the exact order
                # compute consumes them (k-tiles group by group, mask chunks
                # just before the masked tiles t>=20 are reached, later qt
                # columns just before their q-blocks start).
                def ktc(t0, t1):
                    nc.sync.dma_start(
                        kt[:, t0 * 128: t1 * 128], kt_d[h, :, t0 * 128: t1 * 128])

                def v1c(t0, t1):
                    nc.sync.dma_start(v1[:, t0:t1, :], v1_d[h, :, t0:t1, :])

                def qtc(c0, c1):
                    nc.sync.dma_start(qt[:, c0:c1], qt_d[h, :, c0:c1])

                def mrc(jj, c0=0, c1=TRI):
                    nc.sync.dma_start(
                        mref_sb[:, jj, c0:c1], mref_d[jj, :, c0:c1])

                # just-in-time order: each chunk lands right before the
                # pipeline consumes it (QK needs kt; the t=4 mask matmul in
                # group 1 needs id+mredux early; PV needs v1 two groups after
                # its exp; mref chunks feed the deferred masked groups)
                ktc(0, 3); qtc(0, 512); ktc(3, 6)
                nc.sync.dma_start(id_sb[:, :], id_d[:, :])
                nc.sync.dma_start(mrdx_sb[:, :], mrdx_d[:, :])
                ktc(6, 10); v1c(0, 3); v1c(3, 6)
                ktc(10, 15); v1c(6, 10)
                qtc(512, 1024)
                # early: only the mask columns qb0/qb1's deferred masked
                # groups read (q < 768); the rest follows after the qt loads
                ktc(15, 20); mrc(0, 0, 768); v1c(10, 15)
                ktc(20, 25); v1c(15, 20); mrc(1, 0, 768)
                mrc(2, 0, 768); v1c(20, 25); mrc(3, 0, 768); mrc(4, 0, 768)
                qtc(1024, 2048); qtc(2048, S)
                for jj in range(5):
                    mrc(jj, 768, TRI)
            else:
                # Chunked too: the DMA queue is effectively serial, so a
                # monolithic 4.5us load would head-of-line block the
                # latency-critical out-stores of the running head.
                for c in range(5):
                    t0, t1 = c * 5, min((c + 1) * 5, NKT)
                    nc.sync.dma_start(
                        kt[:, t0 * 128: t1 * 128], kt_d[h, :, t0 * 128: t1 * 128])
                    nc.sync.dma_start(v1[:, t0:t1, :], v1_d[h, :, t0:t1, :])
                for c in range(4):
                    c0, c1 = c * 792, (c + 1) * 792
                    nc.sync.dma_start(qt[:, c0:c1], qt_d[h, :, c0:c1])
            nc.sync.dma_start(v2[:, :, :], v2_d[h])
            nc.sync.dma_start(v3[:, :, :], v3_d[h])

        # ---- compute: one global 2-deep software pipeline over every
        # (head, segment, group) work item. PE stream is
        #   ..., QK_g, PV_{g-2}, QK_{g+1}, PV_{g-1}, ...
        # so PE never stalls on a just-finished exp, ACT runs back to back,
        # and the pipeline never drains at segment or head boundaries.
        items = []
        out_f = out_d.rearrange("h s d -> (h s) d")
        for h in range(HPC):
            kt, qt, v1 = kt_sb[h], qt_sb[h], v1_sb[h]
            ktr = kt[:]
            qtr = qt[:]

            def norm_out(acc, si, qsw, row0):
                rec = smallp.tile([128, 1], F32, tag="rec")
                nc.vector.reciprocal(
                    rec[0:qsw, :], acc[0:qsw, si * 129 + 128: si * 129 + 129])
                stg = smallp.tile([128, 128], F32, tag="stg")
                nc.vector.tensor_scalar_mul(
                    stg[0:qsw, :],
                    acc[0:qsw, si * 129: si * 129 + 128], rec[0:qsw, :])
                nc.sync.dma_start(out_f[row0: row0 + qsw, :], stg[0:qsw, :])

            # ===== seg1 items =====
            # Head 0 only: emit qb0/qb1's unmasked k-groups before their
            # masked ones so the first masked exp lands after the ~9us of
            # mask DMA has streamed in (mask matmuls commute -- PSUM adds).
            seg1_items = {}
            for qbi, (q0, qw) in enumerate(QBLOCKS):
                subs = _subs(qw)
                blk = {}  # lazily-allocated acc shared by the block's groups

                def qk1(st, h=h, ktr=ktr, qtr=qtr, q0=q0, qw=qw,
                        tiles=None):
                    for j, t in enumerate(tiles):
                        # Masked tiles: PE writes the mask into PSUM first
                        # (identity matmul, start=True clears the bank), then
                        # the QK matmul accumulates on top (start=False).
                        # Keeps DVE out of the exp critical chain.
                        masked = False
                        if t == 4:
                            # redux keys = partitions 0:64 of tile 4; mredux
                            # is zero-padded to the FULL [128, q] region --
                            # the mask matmul must cover the entire QK output
                            # region (partial-region PSUM init followed by an
                            # accumulating matmul corrupts the uncovered
                            # cells).
                            if q0 + qw > TRE:
                                # lhsT = identity rows 0:64 x all 128 cols:
                                # out rows 0:64 get the mask, rows 64:128 get
                                # an explicit 0 -- full-region coverage with
                                # half the mask bytes.
                                nc.tensor.matmul(
                                    st[:, j, 0:qw],
                                    lhsT=id_sb[0:64, :],
                                    rhs=mrdx_sb[:, q0:q0 + qw],
                                    start=True, stop=False)
                                masked = True
                        elif t >= 20:
                            nc.tensor.matmul(
                                st[:, j, 0:qw],
                                lhsT=id_sb[:, :],
                                rhs=mref_sb[:, t - 20, q0:q0 + qw],
                                start=True, stop=False)
                            masked = True
                        nc.tensor.matmul(
                            st[:, j, :qw],
                            lhsT=ktr[:, t * 128:(t + 1) * 128],
                            rhs=qtr[:, q0:q0 + qw],
                            start=not masked, stop=True)

                def ex1(st, pt, qw=qw, ntile=None):
                    nc.scalar.activation(
                        pt[:, 0:ntile, 0:qw], st[:, 0:ntile, 0:qw], EXP)

                def pv1(pt, h=h, v1=v1, q0=q0, qw=qw, subs=subs, blk=blk,
                        tiles=None):
                    if "acc" not in blk:
                        blk["acc"] = accp.tile([128, 512], F32, tag="acc", name="acc")
                    acc = blk["acc"]
                    for j, t in enumerate(tiles):
                        for si, (qs0, qsw) in enumerate(subs):
                            # start=True clears has_written for the WHOLE
                            # bank, so only the very first matmul into this
                            # bank sets it; other sub-regions overwrite fresh
                            # (their bits are clear) and accumulate after.
                            nc.tensor.matmul(
                                acc[0:qsw, si * 129: si * 129 + 129],
                                lhsT=pt[:, j, qs0:qs0 + qsw],
                                rhs=v1[:, t, :],
                                start=(t == 0 and si == 0),
                                stop=(t == NKT - 1))
                    if NKT - 1 in tiles:
                        for si, (qs0, qsw) in enumerate(subs):
                            norm_out(acc, si, qsw, h * S + q0 + qs0)

                for gi, tiles in enumerate(KGROUPS):
                    seg1_items[(qbi, gi)] = (
                        (lambda st, f=qk1, tl=tiles: f(st, tiles=tl)),
                        (lambda st, pt, f=ex1, n=len(tiles): f(st, pt, ntile=n)),
                        (lambda pt, f=pv1, tl=tiles: f(pt, tiles=tl)),
                    )
            if h == 0:
                # groups 0-3 touch no mref-masked tile; defer the rest of
                # qb0/qb1 until the ref-mask chunks have streamed in
                order = ([(0, gi) for gi in range(4)]
                         + [(1, gi) for gi in range(4)]
                         + [(0, gi) for gi in range(4, 9)]
                         + [(1, gi) for gi in range(4, 9)]
                         + [(qbi, gi) for qbi in range(2, len(QBLOCKS))
                            for gi in range(9)])
            else:
                order = [(qbi, gi) for qbi in range(len(QBLOCKS))
                         for gi in range(9)]
            head_items = [seg1_items[key] for key in order]
            seg23_items = []

            # ===== seg2 items: per-cond ref self-attention =====
            for c in range(NCOND):
                b0 = TRI + 256 * c

                def qk2(st, ktr=ktr, qtr=qtr, b0=b0):
                    for j in range(2):
                        nc.tensor.matmul(
                            st[:, j, 0:256],
                            lhsT=ktr[:, b0 + j * 128: b0 + (j + 1) * 128],
                            rhs=qtr[:, b0: b0 + 256],
                            start=True, stop=True)

                def ex2(st, pt):
                    nc.scalar.activation(
                        pt[:, 0:2, 0:256], st[:, 0:2, 0:256], EXP)

                def pv2(pt, h=h, v2=v2_sb[h], b0=b0, c=c):
                    acc = accp.tile([128, 512], F32, tag="acc", name="acc")
                    for j in range(2):
                        for si in range(2):
                            nc.tensor.matmul(
                                acc[0:128, si * 129: si * 129 + 129],
                                lhsT=pt[:, j, si * 128:(si + 1) * 128],
                                rhs=v2[:, 2 * c + j, :],
                                start=(j == 0 and si == 0), stop=(j == 1))
                    for si in range(2):
                        norm_out(acc, si, 128, h * S + b0 + si * 128)

                seg23_items.append((qk2, ex2, pv2))

            # ===== seg3 item: router queries =====
            def qk3(st, ktr=ktr, qtr=qtr):
                for i in range(16):
                    nc.tensor.matmul(
                        st[:, 0, i * 32:(i + 1) * 32],
                        lhsT=ktr[:, TRE + i * 128: TRE + (i + 1) * 128],
                        rhs=qtr[:, TRIR: TRIR + 32],
                        start=True, stop=True)
                nc.tensor.matmul(
                    st[0:32, 1, 0:32],
                    lhsT=ktr[:, TRIR: TRIR + 32],
                    rhs=qtr[:, TRIR: TRIR + 32],
                    start=True, stop=True)

            def ex3(st, pt):
                nc.scalar.activation(pt[:, 0, 0:512], st[:, 0, 0:512], EXP)
                nc.scalar.activation(pt[0:32, 1, 0:32], st[0:32, 1, 0:32], EXP)

            def pv3(pt, h=h, v3=v3_sb[h]):
                acc = accp.tile([128, 512], F32, tag="acc", name="acc")
                for i in range(16):
                    nc.tensor.matmul(
                        acc[0:32, 0:129],
                        lhsT=pt[:, 0, i * 32:(i + 1) * 32],
                        rhs=v3[:, i, :],
                        start=(i == 0), stop=False)
                nc.tensor.matmul(
                    acc[0:32, 0:129],
                    lhsT=pt[0:32, 1, 0:32],
                    rhs=v3[0:32, 16, :],
                    start=False, stop=True)
                norm_out(acc, 0, 32, h * S + TRIR)

            seg23_items.append((qk3, ex3, pv3))
            # Splice seg2/seg3 into the middle of the head's seg1 stream:
            # their small, bursty exp/PE windows would otherwise sit at the
            # head boundary and stall ACT there. Spaced out so consecutive
            # small-exp items don't cluster.
            for i, it in enumerate(seg23_items):
                head_items.insert(30 + i * 6, it)
            items.extend(head_items)

        # ---- run the global pipeline
        pending = []
        for (fqk, fex, fpv) in items:
            st = stp.tile([128, GROUP, 512], F32, tag="st", name="st")
            fqk(st)
            while len(pending) >= 2:
                pending.pop(0)[0]()
            pt = ptp.tile([128, GROUP, 512], BF16, tag="pt", name="pt")
            fex(st, pt)
            pending.append(((lambda f=fpv, p=pt: f(p)),))
        while pending:
            pending.pop(0)[0]()

    nc.compile()
    return nc


_NC_CACHE = None


def _get_nc():
    global _NC_CACHE
    if _NC_CACHE is None:
        _NC_CACHE = build_nc()
    return _NC_CACHE


def make_in_maps(query, key, value, ref_mask, routing_map):
    q = np.asarray(query, np.float32)[0] * (1.0 / math.sqrt(D))  # [24,S,128]
    k = np.asarray(key, np.float32)[0]
    v = np.asarray(value, np.float32)[0]
    qt = np.ascontiguousarray(q.transpose(0, 2, 1)).astype(
        ml_dtypes.bfloat16)                                      # [24,128,S]
    kt = np.zeros((H, 128, SP), np.float32)
    kt[:, :, :S] = k.transpose(0, 2, 1)
    kt = kt.astype(ml_dtypes.bfloat16)
    vv = np.zeros((H, SP, 129), np.float32)
    vv[:, :S, :128] = v
    vv[:, :S, 128] = 1.0
    vv = vv.astype(ml_dtypes.bfloat16)
    # pre-tiled partition-major V layouts: [H, 128, T, 129]
    v1 = np.ascontiguousarray(
        vv.reshape(H, NKT, 128, 129).transpose(0, 2, 1, 3))
    v2 = np.ascontiguousarray(
        vv[:, TRI:TRIR].reshape(H, 4, 128, 129).transpose(0, 2, 1, 3))
    v3 = np.zeros((H, 17, 128, 129), np.float32).astype(ml_dtypes.bfloat16)
    v3[:, 0:16] = vv[:, TRE:TRI].reshape(H, 16, 128, 129)
    v3[:, 16, 0:32] = vv[:, TRIR:TRIR + 32]
    v3 = np.ascontiguousarray(v3.transpose(0, 2, 1, 3))

    rm = np.asarray(ref_mask, np.float32)[0]                     # [512, 2624]
    rt = np.asarray(routing_map, np.float32)[0]                  # [2, 2048]
    base = (rm - 1.0) * 100.0 + REF_SHIFT
    ref_rt = np.repeat(rt, REF // NCOND, axis=0)                 # [512, 2048]
    base = base.copy()
    base[:, TRE:TRI] += (ref_rt - 1.0) * 100.0
    mref = np.zeros((5, 128, TRI), np.float32)
    mref[0, 64:128] = base[0:64]
    mref[1] = base[64:192]
    mref[2] = base[192:320]
    mref[3] = base[320:448]
    mref[4, 0:64] = base[448:512]
    mref[4, 64:128] = NEG                                        # kill router+pad
    mref = mref.astype(ml_dtypes.bfloat16)
    mredux = np.zeros((64, TRI), np.float32)
    mredux[:, TRE:TRI] = (np.repeat(rt, REDUX // NCOND, axis=0) - 1.0) * 100.0
    mredux = mredux.astype(ml_dtypes.bfloat16)

    ident = np.eye(128, dtype=np.float32).astype(ml_dtypes.bfloat16)
    in_maps = []
    for c in range(8):
        hs = slice(HPC * c, HPC * (c + 1))
        in_maps.append({
            "qt": np.ascontiguousarray(qt[hs]),
            "kt": np.ascontiguousarray(kt[hs]),
            "v1": np.ascontiguousarray(v1[hs]),
            "v2": np.ascontiguousarray(v2[hs]),
            "v3": np.ascontiguousarray(v3[hs]),
            "mref": mref,
            "mredux": np.ascontiguousarray(mredux),
            "ident": ident,
        })
    return in_maps


def kernel(query, key, value, ref_mask, routing_map, **_ignored):
    import jax
    if not any(d.platform == "axon" for d in jax.devices()):
        # the SPMD runner needs the 8 axon-tunneled NeuronCores visible
        jax.config.update("jax_platforms", "axon,cpu")
    nc = _get_nc()
    in_maps = make_in_maps(query, key, value, ref_mask, routing_map)
    res = run_bass_kernel_spmd(nc, in_maps, core_ids=list(range(8)))
    outs = [res.results[c]["out"] for c in range(8)]             # [3,S,128] each
    full = np.concatenate(outs, axis=0)[None]                    # [1,24,S,128]
    return np.ascontiguousarray(full.astype(np.float32))
